# revision 27
# baseline (speedup 1.0000x reference)
"""Self-contained Trainium2 Bass kernel for causal GQA self-attention.

Problem (hardcoded): B=2, T=2048, D=2048, H=16 Q-heads, KV=4 kv-heads,
hd=128, rotate-half RoPE (theta=10000), causal softmax, out-projection.

Distribution over 8 NeuronCores (tensor parallel over heads):
 - core c owns Q heads {2c, 2c+1} and KV head c//2 (each KV head is shared
   by two cores, so the K/V projection is computed twice - cheaper than a
   collective at the start).
 - each core computes q/k/v projections + RoPE + causal attention for its
   heads over BOTH batch rows, entirely locally (bf16 matmuls, f32 softmax).
 - an 8-way AllToAll (two per local head, split by t-halves) reshards the
   attention outputs: afterwards core c holds all 16 heads' outputs for its
   slice of the flattened [B*T] row space (rows [512c, 512c+512)).
 - each core computes the final out-projection for its rows with the full
   Wo and returns its [512, 2048] slice; the host concatenates the slices.
"""

import numpy as np
import ml_dtypes

import concourse.bass as bass
import concourse.tile as tile
from concourse import bacc, mybir
from concourse.bass_utils import run_bass_kernel_spmd

BF = mybir.dt.bfloat16
F32 = mybir.dt.float32

B, T, D = 2, 2048, 2048
H, KVH, HD = 16, 4, 128
THETA = 10000.0
NCORES = 8
TT = T // 128           # 16 t-tiles
KD = D // 128           # 16 contraction tiles
NEG = -30000.0

_compiled = None
PHASE_MARKS = []

# test-harness knobs (not used by the grading path)
TRACE = False
TRACE_DIR = None
LAST_RESULT = None


def _ceil_div(a, b):
    return (a + b - 1) // b


def _build():
    nc = bacc.Bacc(
        "TRN2", target_bir_lowering=False, debug=False, num_devices=NCORES
    )

    # ---- I/O ----
    xt0 = nc.dram_tensor("xt0", [D, T], BF, kind="ExternalInput").ap()
    xt1 = nc.dram_tensor("xt1", [D, T], BF, kind="ExternalInput").ap()
    wqkv = nc.dram_tensor("wqkv", [D, 512], BF, kind="ExternalInput").ap()
    wo = nc.dram_tensor("wo", [D, D], BF, kind="ExternalInput").ap()
    cosq = nc.dram_tensor("cosq", [128, TT * 64], BF, kind="ExternalInput").ap()
    sinq = nc.dram_tensor("sinq", [128, TT * 64], BF, kind="ExternalInput").ap()
    cosk = nc.dram_tensor("cosk", [128, TT * 64], BF, kind="ExternalInput").ap()
    sink = nc.dram_tensor("sink", [128, TT * 64], BF, kind="ExternalInput").ap()
    maskq = nc.dram_tensor("maskq", [128, 2048], BF, kind="ExternalInput").ap()
    identin = nc.dram_tensor("identin", [128, 128], BF, kind="ExternalInput").ap()
    out_ext = nc.dram_tensor("out", [512, D], F32, kind="ExternalOutput").ap()

    xts = [xt0, xt1]

    with tile.TileContext(nc) as tc:
        with (
            tc.tile_pool(name="const", bufs=1) as constp,
            tc.tile_pool(name="big", bufs=1) as bigp,
            tc.tile_pool(name="persist", bufs=1) as persist,
            tc.tile_pool(name="work", bufs=2) as work,
            tc.tile_pool(name="combp", bufs=1) as combp,
            tc.tile_pool(name="ptb", bufs=24) as ptb,
            tc.tile_pool(name="aop", bufs=8) as aop,
            tc.tile_pool(name="stg", bufs=4) as stgp,
            tc.tile_pool(name="psS", bufs=5, space="PSUM") as psS,
            tc.tile_pool(name="dram", bufs=1, space="DRAM") as dram,
        ):
            # ---- constants in SBUF ----
            wqkv_sb = constp.tile([128, KD * 512], BF, tag="wqkv")
            cq = constp.tile([128, TT * 64], BF, tag="cq")
            sq = constp.tile([128, TT * 64], BF, tag="sq")
            ck = constp.tile([128, TT * 64], BF, tag="ck")
            sk = constp.tile([128, TT * 64], BF, tag="sk")
            nc.gpsimd.dma_start(cq[:], cosq)
            nc.gpsimd.dma_start(sq[:], sinq)
            nc.gpsimd.dma_start(ck[:], cosk)
            nc.gpsimd.dma_start(sk[:], sink)
            mask_sb = constp.tile([128, 2048], BF, tag="mask")
            nc.gpsimd.dma_start(mask_sb[:], maskq)
            ident_sb = constp.tile([128, 128], BF, tag="ident")
            nc.gpsimd.dma_start(ident_sb[:], identin)

            # persistent attention operands
            qt_all = persist.tile([128, 4 * T], BF, tag="qt")   # slot=(h*2+b)
            kt_all = persist.tile([128, 2 * T], BF, tag="kt")   # per batch
            vaug = persist.tile([128, 2 * TT * 132], BF, tag="vb")  # per batch, [V|1|pad] tiles

            # A2A bounce buffers (DRAM): 2 heads x 2 t-halves
            a2a_in = [
                dram.tile([1024, 512], BF, name=f"ain{h}", tag=f"ain{h}")
                for h in range(2)
            ]
            a2a_out = [
                dram.tile([1024, 512], BF, name=f"aout{h}", tag=f"aout{h}")
                for h in range(2)
            ]
            rg = [list(range(NCORES))]

            wo_holder = []

            def load_wo():
                wo_sb = bigp.tile([128, KD * D], BF, tag="big")
                for i in range(KD):
                    nc.sync.dma_start(
                        wo_sb[:, i * D : (i + 1) * D],
                        wo[i * 128 : (i + 1) * 128, :],
                    )
                wo_holder.append(wo_sb)

            def projections(b):
                """QKV projections + RoPE + q/k transposes for batch b."""
                xt_sb = bigp.tile([128, KD * T], BF, tag="big")
                for kd in range(KD):
                    if b == 0:
                        nc.sync.dma_start(
                            wqkv_sb[:, kd * 512 : (kd + 1) * 512],
                            wqkv[kd * 128 : (kd + 1) * 128, :],
                        )
                    nc.sync.dma_start(
                        xt_sb[:, kd * T : (kd + 1) * T],
                        xts[b][kd * 128 : (kd + 1) * 128, :],
                    )
                comb = combp.tile([128, TT * 512], BF, tag="comb")
                for tt in range(TT):
                    ps = psS.tile([128, 512], F32, tag="s")
                    for kd in range(KD):
                        nc.tensor.matmul(
                            ps[:],
                            xt_sb[:, kd * T + tt * 128 : kd * T + (tt + 1) * 128],
                            wqkv_sb[:, kd * 512 : (kd + 1) * 512],
                            start=(kd == 0),
                            stop=(kd == KD - 1),
                        )
                    nc.scalar.copy(comb[:, tt * 512 : (tt + 1) * 512], ps[:])

                comb3 = comb[:].rearrange("p (i u) -> p i u", u=512)
                # v: strided copy of nat tiles into vaug (+ ones column)
                vb3 = vaug[:, b * TT * 132 : (b + 1) * TT * 132].rearrange(
                    "p (i u) -> p i u", u=132
                )
                cq3 = cq[:].rearrange("p (i u) -> p i u", u=64)
                sq3 = sq[:].rearrange("p (i u) -> p i u", u=64)
                ck3 = ck[:].rearrange("p (i u) -> p i u", u=64)
                sk3 = sk[:].rearrange("p (i u) -> p i u", u=64)
                for g in range(4):
                    gs = slice(g * 4, (g + 1) * 4)
                    nc.vector.tensor_copy(
                        vb3[:, gs, 0:128], comb3[:, gs, 384:512]
                    )
                    nc.vector.memset(vb3[:, gs, 128:129], 1.0)
                    for iu in range(3):
                        c3, s3 = (cq3, sq3) if iu < 2 else (ck3, sk3)
                        lo = comb3[:, gs, iu * 128 : iu * 128 + 64]
                        hi = comb3[:, gs, iu * 128 + 64 : iu * 128 + 128]
                        ro = work.tile([128, 512], BF, tag="rope_out", bufs=3)
                        ro3 = ro[:].rearrange("p (i u) -> p i u", u=128)
                        t1 = work.tile([128, 256], BF, tag="rt1", bufs=2)
                        t2_ = work.tile([128, 256], BF, tag="rt2", bufs=2)
                        t13 = t1[:].rearrange("p (i u) -> p i u", u=64)
                        t23 = t2_[:].rearrange("p (i u) -> p i u", u=64)
                        nc.vector.tensor_mul(t13, lo, c3[:, gs, :])
                        nc.vector.tensor_mul(t23, hi, s3[:, gs, :])
                        nc.vector.tensor_sub(ro3[:, :, 0:64], t13, t23)
                        t3 = work.tile([128, 256], BF, tag="rt3", bufs=2)
                        t4 = work.tile([128, 256], BF, tag="rt4", bufs=2)
                        t33 = t3[:].rearrange("p (i u) -> p i u", u=64)
                        t43 = t4[:].rearrange("p (i u) -> p i u", u=64)
                        nc.vector.tensor_mul(t33, hi, c3[:, gs, :])
                        nc.vector.tensor_mul(t43, lo, s3[:, gs, :])
                        nc.vector.tensor_add(ro3[:, :, 64:128], t33, t43)
                        if iu < 2:
                            dst, off = qt_all, (iu * 2 + b) * T
                        else:
                            dst, off = kt_all, b * T
                        tps = psS.tile([128, 512], BF, tag="tp", bufs=1)
                        for i in range(4):
                            nc.tensor.transpose(
                                tps[:, i * 128 : (i + 1) * 128],
                                ro[:, i * 128 : (i + 1) * 128],
                                ident_sb[:],
                            )
                        nc.vector.tensor_copy(
                            dst[:, off + g * 512 : off + (g + 1) * 512], tps[:]
                        )

            def attention(h, b, ao_t):
                """Causal attention for local head h, batch b (S^T form).

                Writes attnout^T [hd, T] into ao_t columns [b*T, (b+1)*T).
                """
                slot = h * 2 + b
                ao_nat = work.tile([128, T], BF, tag="aonat")

                def s_blocks(quad, jlo, jhi):
                    t0 = quad * 4
                    q0 = slot * T + quad * 512
                    out = []
                    for j in range(jlo, jhi):
                        m = j - t0
                        c0 = max(m, 0) * 128
                        w = 512 - c0
                        sps = psS.tile([128, 512], F32, tag="s")
                        nc.tensor.matmul(
                            sps[:, 0:w],
                            kt_all[:, b * T + j * 128 : b * T + (j + 1) * 128],
                            qt_all[:, q0 + c0 : q0 + 512],
                            start=True,
                            stop=True,
                        )
                        pb = ptb.tile([128, 512], BF, tag="pb")
                        nc.scalar.activation(
                            pb[:, c0:512], sps[:, 0:w],
                            mybir.ActivationFunctionType.Exp,
                            bias=0.0, scale=1.0,
                        )
                        if m >= 0:
                            nc.vector.tensor_mul(
                                pb[:, c0:512], pb[:, c0:512],
                                mask_sb[:, m * 512 + c0 : (m + 1) * 512],
                            )
                        out.append(pb)
                    return out

                blocks = {0: s_blocks(0, 0, 4)}
                for quad in range(4):
                    t0 = quad * 4
                    # lookahead: emit next quad's first S blocks before this AV
                    if quad < 3:
                        blocks[quad + 1] = s_blocks(quad + 1, 0, 8)
                    for i in range(4):
                        tau = t0 + i
                        avps = psS.tile([128, 132], F32, tag="av", bufs=2)
                        for j in range(tau + 1):
                            nc.tensor.matmul(
                                avps[:, 0:129],
                                blocks[quad][j][:, i * 128 : (i + 1) * 128],
                                vaug[
                                    :,
                                    b * TT * 132 + j * 132 : b * TT * 132 + j * 132 + 129,
                                ],
                                start=(j == 0),
                                stop=(j == tau),
                            )
                        r = stgp.tile([128, 1], F32, tag="rc", bufs=4)
                        nc.vector.reciprocal(r[:], avps[:, 128:129])
                        nc.vector.tensor_scalar_mul(
                            ao_nat[:, tau * 128 : (tau + 1) * 128],
                            avps[:, 0:128],
                            r[:],
                        )
                    if quad < 3:
                        blocks[quad + 1].extend(
                            s_blocks(quad + 1, 8, (quad + 1) * 4 + 4)
                        )
                    del blocks[quad]
                # transpose attnout nat [t, hd] -> [hd, t]
                for i0 in range(0, TT, 4):
                    tps = psS.tile([128, 512], BF, tag="tp", bufs=1)
                    for i in range(i0, i0 + 4):
                        nc.tensor.transpose(
                            tps[:, (i - i0) * 128 : (i - i0 + 1) * 128],
                            ao_nat[:, i * 128 : (i + 1) * 128],
                            ident_sb[:],
                        )
                    last = nc.vector.tensor_copy(
                        ao_t[:, b * T + i0 * 128 : b * T + (i0 + 4) * 128], tps[:]
                    )
                return last

            def stage_a2a(h, ao):
                """Write attnout^T slices into the A2A input bounce buffer."""
                for js in range(8):
                    bb = js // 4
                    q4 = js % 4
                    nc.sync.dma_start(
                        a2a_in[h][js * 128 : (js + 1) * 128, :],
                        ao[:, bb * T + q4 * 512 : bb * T + (q4 + 1) * 512],
                    )

            def fire_a2a(h):
                nc.gpsimd.collective_compute(
                    "AllToAll",
                    mybir.AluOpType.bypass,
                    replica_groups=rg,
                    ins=[a2a_in[h].opt()],
                    outs=[a2a_out[h].opt()],
                )

            def oproj_chunk(h, after=None):
                """Out-projection rows for this core using gathered head h."""
                wo_sb = wo_holder[0]
                aos = []
                for r in range(8):
                    t = aop.tile([128, 512], BF, tag="aotile")
                    ld = nc.sync.dma_start(
                        t[:], a2a_out[h][r * 128 : (r + 1) * 128, :]
                    )
                    if after is not None:
                        tile.add_dep_helper(
                            ld.ins,
                            after.ins,
                            sync=False,
                            reason="schedule oproj after attn11",
                        )
                    aos.append(t)
                for tt2 in range(4):
                    row0 = tt2 * 128
                    for dc in range(4):
                        ops = psS.tile([128, 512], F32, tag="s")
                        for r in range(8):
                            head = 2 * r + h
                            nc.tensor.matmul(
                                ops[:],
                                aos[r][:, tt2 * 128 : (tt2 + 1) * 128],
                                wo_sb[:, head * D + dc * 512 : head * D + (dc + 1) * 512],
                                start=(r == 0),
                                stop=(r == 7),
                            )
                        stg = stgp.tile([128, 512], F32, tag="ostage", bufs=2)
                        nc.scalar.copy(stg[:], ops[:])
                        nc.gpsimd.dma_start(
                            out_ext[row0 : row0 + 128, dc * 512 : (dc + 1) * 512],
                            stg[:],
                            accum_op=(
                                mybir.AluOpType.bypass
                                if h == 0
                                else mybir.AluOpType.add
                            ),
                        )

            # ---- main schedule ----
            PHASE_MARKS.append(("proj0", nc.next_id()))
            with nc.named_scope("proj0"):
                projections(0)
            ao0 = work.tile([128, 2 * T], BF, tag="atout")
            PHASE_MARKS.append(("attn00", nc.next_id()))
            with nc.named_scope("attn00"):
                attention(0, 0, ao0)
            PHASE_MARKS.append(("proj1", nc.next_id()))
            with nc.named_scope("proj1"):
                projections(1)
                load_wo()
            PHASE_MARKS.append(("attn01", nc.next_id()))
            with nc.named_scope("attn01"):
                attention(0, 1, ao0)
            PHASE_MARKS.append(("a2a0", nc.next_id()))
            with nc.named_scope("a2a0"):
                stage_a2a(0, ao0)
                fire_a2a(0)
            ao1 = work.tile([128, 2 * T], BF, tag="atout")
            PHASE_MARKS.append(("attn10", nc.next_id()))
            with nc.named_scope("attn10"):
                attention(1, 0, ao1)
            PHASE_MARKS.append(("attn11", nc.next_id()))
            with nc.named_scope("attn11"):
                attn11_last = attention(1, 1, ao1)
            PHASE_MARKS.append(("a2a1", nc.next_id()))
            with nc.named_scope("a2a1"):
                stage_a2a(1, ao1)
                fire_a2a(1)
            PHASE_MARKS.append(("oproj0", nc.next_id()))
            with nc.named_scope("oproj0"):
                oproj_chunk(0)
            PHASE_MARKS.append(("oproj1", nc.next_id()))
            with nc.named_scope("oproj1"):
                oproj_chunk(1)

    PHASE_MARKS.append(("end", nc.next_id()))
    nc.compile()
    return nc


def _get_compiled():
    global _compiled
    if _compiled is None:
        _compiled = _build()
    return _compiled


def _rope_tables():
    """Natural-layout RoPE tables [128, TT*64] (t-tile-major blocks)."""
    inv_freq = 1.0 / (THETA ** (np.arange(0, HD, 2, dtype=np.float64) / HD))  # [64]
    t = np.arange(T, dtype=np.float64)
    ang = t[:, None] * inv_freq[None, :]          # [T, 64]
    cos = np.cos(ang).astype(np.float32)
    sin = np.sin(ang).astype(np.float32)
    # [T, 64] -> [128, TT*64]: block i columns = rows [128i, 128(i+1))
    cos_n = cos.reshape(TT, 128, 64).transpose(1, 0, 2).reshape(128, TT * 64)
    sin_n = sin.reshape(TT, 128, 64).transpose(1, 0, 2).reshape(128, TT * 64)
    return cos_n, sin_n


def kernel(x, Wq, Wk, Wv, Wo):
    x = np.asarray(x)
    Wq_ = np.asarray(Wq)
    Wk_ = np.asarray(Wk)
    Wv_ = np.asarray(Wv)
    Wo_ = np.asarray(Wo)

    bf = ml_dtypes.bfloat16
    xt = [np.ascontiguousarray(x[b].T).astype(bf) for b in range(B)]
    wo_bf = Wo_.astype(bf)

    cos_n, sin_n = _rope_tables()
    scale = 1.0 / np.sqrt(np.float32(HD))
    cosq = (cos_n * scale).astype(bf)
    sinq = (sin_n * scale).astype(bf)
    cosk = cos_n.astype(bf)
    sink = sin_n.astype(bf)

    kl = np.arange(128)[:, None]
    ql = np.arange(512)[None, :]
    maskq = np.concatenate(
        [(ql >= kl + m * 128).astype(np.float32) for m in range(4)], axis=1
    ).astype(bf)

    in_maps = []
    for c in range(NCORES):
        kv = c // 2
        wqkv = np.concatenate(
            [
                Wq_[:, 2 * c * 128 : (2 * c + 2) * 128],
                Wk_[:, kv * 128 : (kv + 1) * 128],
                Wv_[:, kv * 128 : (kv + 1) * 128],
            ],
            axis=1,
        ).astype(bf)
        in_maps.append(
            {
                "xt0": xt[0],
                "xt1": xt[1],
                "wqkv": wqkv,
                "wo": wo_bf,
                "cosq": cosq,
                "sinq": sinq,
                "cosk": cosk,
                "sink": sink,
                "maskq": maskq,
                "identin": np.eye(128, dtype=np.float32).astype(bf),
            }
        )

    nc = _get_compiled()
    global LAST_RESULT
    kw = {}
    if TRACE:
        kw = dict(trace=True, tmpdir=TRACE_DIR)
    res = run_bass_kernel_spmd(nc, in_maps, list(range(NCORES)), **kw)
    LAST_RESULT = res
    out = np.empty((B * T, D), dtype=np.float32)
    for c in range(NCORES):
        out[c * 512 : (c + 1) * 512, :] = res.results[c]["out"]
    return out.reshape(B, T, D)
